# revision 18
# baseline (speedup 1.0000x reference)
"""GNN message-passing kernel for 8 Trainium2 NeuronCores.

Reference computation:
    t   = node_feats @ W + b                       # [N, H]
    msgs = t[nbr] + edge_feats[eid]                # [E, H]
    agg = segment_sum(msgs, dst, N)                # [N, H]
    out = t + agg

Sharding: dst is sorted, so core k owns the node range [k*6250, (k+1)*6250)
and the contiguous edge slice whose dst falls in that range. No collectives:
by linearity, sum_e t[nbr_e] = (sum_e nf[nbr_e]) @ W + deg*b, so each core
gathers raw fp8 node features from a replicated table and applies W once per
window after aggregation:
    out = (nf_own + sum nf[nbr]) @ W + (1 + deg) * b + sum ef

Per core, edges are grouped into 49 windows of 128 dst nodes, each split in
two gather groups by neighbor id (A: nbr < 25000, B: rest) so gather indices
fit int16. Slot columns are laid out A-blocks first, then B-blocks, and the
gathers are merged: one dma_gather covers a CHUNK of consecutive windows'
A (or B) columns, cutting the Pool-engine descriptor-generation serial cost
~4x versus per-window calls. Pad slots carry index 0 (a real row, so no NaNs
ever enter SBUF and no memsets or count registers are needed); the one-hot
rows of pad slots are zero, which nulls their contribution.

Each window accumulates:
  psa[f_lo, d] += g[:, cH:cH+128].T @ onehot ; self[:, :128].T @ I   (fp8)
  psb[f_hi, d] += g[:, cH+128:+256].T @ onehot ; self half           (fp8)
  pso[d, h]    += onehot.T @ ef_tile  + (1+deg).T @ b
then the W-transform: copy psa/psb to SBUF bf16 S_T and
  pso += S_T.T @ W  (two K=128 bf16 matmuls), flush pso to HBM.
"""

import sys

sys.path.insert(0, "/opt/trn_rl_repo")

import ml_dtypes
import numpy as np

import concourse.bacc as bacc
import concourse.mybir as mybir
import concourse.tile as tile
from concourse.bass_utils import run_bass_kernel_spmd
from concourse.library_config import mlp

N_NODES = 50000
N_EDGES = 800000
H = 256
N_CORES = 8
NODES_PER_CORE = N_NODES // N_CORES          # 6250
WIN = 128                                    # dst nodes per PSUM window
N_WIN = (NODES_PER_CORE + WIN - 1) // WIN    # 49 (last window = 106 nodes)
SPLIT = 25000                                # A: nbr < SPLIT, B: rest
CHUNK = 4                                    # windows per merged gather call

F8 = mybir.dt.float8e3                       # e3m4: 4 mantissa bits
F8NP = ml_dtypes.float8_e3m4

_cache = {}
_last_in_maps = None


def _build_schedule(dst, nbr):
    """Host-side slot schedule, shared shapes across cores (single NEFF).

    Column space: A-group columns of all windows first (c0[w, 0]), then all
    B-group columns (c0[w, 1]). Returns (T, c0, S, per_core).
    """
    bounds = np.searchsorted(dst, np.arange(N_CORES + 1) * NODES_PER_CORE)
    counts = np.zeros((N_CORES, N_WIN, 2), dtype=np.int64)
    per_core_raw = []
    for k in range(N_CORES):
        e0, e1 = bounds[k], bounds[k + 1]
        dk = dst[e0:e1].astype(np.int64) - k * NODES_PER_CORE
        nk = nbr[e0:e1].astype(np.int64)
        w = dk >> 7
        q = (nk >= SPLIT).astype(np.int64)
        key = q * N_WIN + w
        np.add.at(counts[k], (w, q), 1)
        per_core_raw.append((e0, e1, dk, nk, key))

    T = np.maximum((counts.max(axis=0) + 127) // 128, 1)   # [N_WIN, 2]
    c0 = np.zeros((N_WIN, 2), dtype=np.int64)
    s = 0
    for q in range(2):
        for w in range(N_WIN):
            c0[w, q] = s
            s += T[w, q]
    S = int(s)

    per_core = []
    for k in range(N_CORES):
        e0, e1, dk, nk, key = per_core_raw[k]
        # sort slots by nbr within each group: ascending gather addresses
        order = np.lexsort((nk, key))
        sorted_key = key[order]
        group_start = np.searchsorted(sorted_key, np.arange(N_WIN * 2))
        j_within = np.arange(len(order)) - group_start[sorted_key]
        slot = np.empty(len(order), dtype=np.int64)
        slot[order] = j_within
        base_col = c0[key % N_WIN, key // N_WIN]
        p = slot % 128
        c = base_col + slot // 128
        per_core.append((e0, e1, dk, nk, p, c))
    return T, c0, S, per_core


def _build_program(T, c0, S):
    nc = bacc.Bacc(num_swdge_queues=4)
    f32, bf16, i16 = mybir.dt.float32, mybir.dt.bfloat16, mybir.dt.int16

    nf8 = nc.declare_dram_parameter("nf8", [N_NODES, H], F8, isOutput=False)
    identp = nc.declare_dram_parameter("ident", [128, 128], F8, isOutput=False)
    W16 = nc.declare_dram_parameter("W16", [H, H], bf16, isOutput=False)
    b16p = nc.declare_dram_parameter("b16", [1, H], bf16, isOutput=False)
    deg1 = nc.declare_dram_parameter("deg1", [1, N_WIN * WIN], bf16, isOutput=False)
    self8p = nc.declare_dram_parameter("self8", [128, N_WIN * H], F8, isOutput=False)
    ef = nc.declare_dram_parameter("ef", [128, S, H], F8, isOutput=False)
    ohp = nc.declare_dram_parameter("oh", [128, S * 128], F8, isOutput=False)
    gidx = nc.declare_dram_parameter("gidx", [128, 8 * S], i16, isOutput=False)
    outp = nc.declare_dram_parameter("out", [NODES_PER_CORE, H], f32, isOutput=True)

    n_chunks = (N_WIN + CHUNK - 1) // CHUNK
    # columns covered by chunk i, group q: [cc0[i][q], cc1[i][q])
    cc0 = [[int(c0[i * CHUNK, q]) for q in range(2)] for i in range(n_chunks)]
    cc1 = [
        [
            int(
                c0[min((i + 1) * CHUNK, N_WIN) - 1, q]
                + T[min((i + 1) * CHUNK, N_WIN) - 1, q]
            )
            for q in range(2)
        ]
        for i in range(n_chunks)
    ]
    camax = max(cc1[i][0] - cc0[i][0] for i in range(n_chunks))
    cbmax = max(cc1[i][1] - cc0[i][1] for i in range(n_chunks))
    LAG = 2 * CHUNK

    with tile.TileContext(nc) as tc:
        nc.gpsimd.load_library(mlp)
        with (
            tc.tile_pool(name="const", bufs=1) as cpool,
            tc.tile_pool(name="psA", bufs=2, space="PSUM") as ppa,
            tc.tile_pool(name="psB", bufs=2, space="PSUM") as ppb,
            tc.tile_pool(name="psO", bufs=3, space="PSUM") as ppo,
            tc.tile_pool(name="gathA", bufs=3) as gpa,
            tc.tile_pool(name="gathB", bufs=3) as gpb,
            tc.tile_pool(name="ef8", bufs=4) as e8p,
            tc.tile_pool(name="oneh", bufs=4) as ohpool,
            tc.tile_pool(name="sT", bufs=3) as stp,
            tc.tile_pool(name="flush", bufs=3) as flp,
        ):
            # resident constants; gidx first so gathers can start early
            gidx_s = cpool.tile([128, 8 * S], i16)
            third = ((8 * S) // 3) & ~7
            nc.sync.dma_start(out=gidx_s[:, :third], in_=gidx[:, :third])
            nc.sync.dma_start(out=gidx_s[:, third:2 * third], in_=gidx[:, third:2 * third])
            nc.sync.dma_start(out=gidx_s[:, 2 * third:], in_=gidx[:, 2 * third:])
            id8 = cpool.tile([128, 128], F8)
            nc.scalar.dma_start(out=id8[:], in_=identp[:])
            w16 = cpool.tile([128, 2 * H], bf16)           # W in two K-halves
            nc.scalar.dma_start(out=w16[:, :H], in_=W16[0:128, :])
            nc.scalar.dma_start(out=w16[:, H:], in_=W16[128:256, :])
            b16 = cpool.tile([1, H], bf16)
            nc.scalar.dma_start(out=b16[:], in_=b16p[:])
            d16 = cpool.tile([1, N_WIN * WIN], bf16)
            nc.scalar.dma_start(out=d16[:], in_=deg1[:])
            self_s = cpool.tile([128, N_WIN * H], F8)
            nc.scalar.dma_start(out=self_s[:], in_=self8p[:])

            g_tiles = {}
            eo_tiles = {}

            def issue_gathers(i):
                for q, (pool, wmax) in enumerate(((gpa, camax), (gpb, cbmax))):
                    lo, hi = cc0[i][q], cc1[i][q]
                    ncols = hi - lo
                    g = pool.tile([128, wmax * H], F8, tag=f"g{q}")
                    g_tiles[(i, q)] = g
                    src = nf8[0:32768, :] if q == 0 else nf8[SPLIT:N_NODES, :]
                    nc.gpsimd.dma_gather(
                        out_ap=g[:, :ncols * H].rearrange("p (c d) -> p c d", d=H),
                        in_ap=src,
                        idxs_ap=gidx_s[:, 8 * lo: 8 * hi],
                        num_idxs=ncols * 128,
                        num_idxs_reg=ncols * 128,
                        elem_size=H,
                        single_packet=False,
                        queue_num=(2 * i + q) % 4,
                    )

            def issue_streams(w):
                res = []
                for q in range(2):
                    tw = int(T[w, q])
                    cc = int(c0[w, q])
                    e8 = e8p.tile([128, tw * H], F8, tag=f"ef{q}")
                    nc.sync.dma_start(
                        out=e8[:].rearrange("p (c d) -> p c d", d=H),
                        in_=ef[:, cc:cc + tw, :],
                    )
                    oh = ohpool.tile([128, tw * 128], F8, tag=f"oh{q}")
                    nc.scalar.dma_start(out=oh[:], in_=ohp[:, cc * 128:(cc + tw) * 128])
                    res.append((e8, oh))
                eo_tiles[w] = res

            pend = {}

            def compute_ef(w):
                n0 = w * WIN
                psa = ppa.tile([128, 128], f32, tag="psa")
                psb = ppb.tile([128, 128], f32, tag="psb")
                pso = ppo.tile([128, H], f32, tag="pso")
                eo = eo_tiles.pop(w)
                # self term opens the psT groups; bias opens pso
                sl = self_s[:, w * H:(w + 1) * H]
                nc.tensor.matmul(psa[:], lhsT=sl[:, :128], rhs=id8[:], start=True, stop=False)
                nc.tensor.matmul(psb[:], lhsT=sl[:, 128:], rhs=id8[:], start=True, stop=False)
                nc.tensor.matmul(pso[:], lhsT=d16[:, n0:n0 + WIN], rhs=b16[:], start=True, stop=False)
                for q in range(2):
                    tw = int(T[w, q])
                    e8, oh = eo[q]
                    for c in range(tw):
                        ohc = oh[:, c * 128:(c + 1) * 128]
                        nc.tensor.matmul(pso[:], lhsT=ohc, rhs=e8[:, c * H:(c + 1) * H],
                                         start=False, stop=False)
                pend[w] = (psa, psb, pso, eo)

            def compute_g(w):
                psa, psb, pso, eo = pend[w]
                i = w // CHUNK
                for q in range(2):
                    tw = int(T[w, q])
                    _, oh = eo[q]
                    g = g_tiles[(i, q)]
                    goff = int(c0[w, q]) - cc0[i][q]
                    for c in range(tw):
                        ohc = oh[:, c * 128:(c + 1) * 128]
                        gc = g[:, (goff + c) * H:(goff + c + 1) * H]
                        last = q == 1 and c == tw - 1
                        nc.tensor.matmul(psa[:], lhsT=gc[:, :128], rhs=ohc,
                                         start=False, stop=last)
                        nc.tensor.matmul(psb[:], lhsT=gc[:, 128:], rhs=ohc,
                                         start=False, stop=last)

            def transform(w):
                psa, psb, pso, _ = pend.pop(w)
                n0 = w * WIN
                nn = min(WIN, NODES_PER_CORE - n0)
                st = stp.tile([128, H], bf16, tag="sT")
                nc.vector.tensor_copy(out=st[:, :128], in_=psa[:])
                nc.vector.tensor_copy(out=st[:, 128:], in_=psb[:])
                nc.tensor.matmul(pso[:], lhsT=st[:, :128], rhs=w16[:, :H],
                                 start=False, stop=False)
                nc.tensor.matmul(pso[:], lhsT=st[:, 128:], rhs=w16[:, H:],
                                 start=False, stop=True)
                fl = flp.tile([128, H], f32, tag="flush")
                nc.scalar.copy(out=fl[:nn, :], in_=pso[:nn, :])
                nc.scalar.dma_start(out=outp[n0:n0 + nn, :], in_=fl[:nn, :])

            for step in range(N_WIN + LAG):
                if step < N_WIN and step % CHUNK == 0:
                    issue_gathers(step // CHUNK)
                ws = step - (LAG - 3)
                if 0 <= ws < N_WIN:
                    issue_streams(ws)
                if step >= LAG:
                    w = step - LAG
                    compute_ef(w)
                    if w > 0:
                        transform(w - 1)
                    compute_g(w)
            transform(N_WIN - 1)

    nc.compile()
    return nc


def kernel(node_feats, edge_feats, W, b, dst, nbr, eid):
    global _last_in_maps
    node_feats = np.ascontiguousarray(np.asarray(node_feats, dtype=np.float32))
    edge_feats = np.ascontiguousarray(np.asarray(edge_feats, dtype=np.float32))
    W = np.ascontiguousarray(np.asarray(W, dtype=np.float32))
    b = np.asarray(b, dtype=np.float32).reshape(1, H)
    dst = np.asarray(dst, dtype=np.int32)
    nbr = np.asarray(nbr, dtype=np.int32)
    eid = np.asarray(eid, dtype=np.int32)

    T, c0, S, per_core = _build_schedule(dst, nbr)

    key = (S, T.tobytes())
    if key not in _cache:
        _cache.clear()
        _cache[key] = _build_program(T, c0, S)
    nc = _cache[key]

    bf = ml_dtypes.bfloat16
    nf8_arr = node_feats.astype(F8NP)
    ident = np.zeros((128, 128), dtype=F8NP)
    np.fill_diagonal(ident, F8NP(1.0))
    W16 = W.astype(bf)
    b16 = b.astype(bf)

    n_chunks = (N_WIN + CHUNK - 1) // CHUNK
    # column -> gather-call base column (for relative j encoding)
    callbase = np.zeros(S, dtype=np.int64)
    for i in range(n_chunks):
        w0 = i * CHUNK
        w1 = min(w0 + CHUNK, N_WIN)
        for q in range(2):
            lo = c0[w0, q]
            hi = c0[w1 - 1, q] + T[w1 - 1, q]
            callbase[lo:hi] = lo

    in_maps = []
    for k in range(N_CORES):
        e0, e1, dk, nk, p, c = per_core[k]
        ef_arr = np.zeros((128, S, H), dtype=F8NP)
        ef_arr[p, c] = edge_feats[eid[e0:e1]].astype(F8NP)
        oh_arr = np.zeros((128, S, 128), dtype=F8NP)
        oh_arr[p, c, dk & 127] = F8NP(1.0)
        # gather indices: call-relative slot j = (c - callbase)*128 + p,
        # encoded at [j%16, 8*callbase + j//16]; pad slots keep index 0
        gidx_arr = np.zeros((16, 8 * S), dtype=np.int16)
        q_arr = (nk >= SPLIT).astype(np.int64)
        idx_val = np.where(q_arr == 0, nk, nk - SPLIT)
        base_col = callbase[c]
        j = (c - base_col) * 128 + p
        gidx_arr[j % 16, 8 * base_col + j // 16] = idx_val.astype(np.int16)
        gidx_full = np.tile(gidx_arr, (8, 1))
        w_arr = dk >> 7
        deg1_arr = np.zeros((1, N_WIN * WIN), dtype=np.float32)
        deg1_arr[0, :NODES_PER_CORE] = 1.0
        np.add.at(deg1_arr[0], dk, 1.0)
        own8 = np.zeros((N_WIN * WIN, H), dtype=F8NP)
        own8[:NODES_PER_CORE] = nf8_arr[k * NODES_PER_CORE:(k + 1) * NODES_PER_CORE]
        self8_arr = np.ascontiguousarray(
            own8.reshape(N_WIN, WIN, H).transpose(1, 0, 2).reshape(WIN, N_WIN * H)
        )
        in_maps.append({
            "nf8": nf8_arr,
            "ident": ident,
            "W16": W16,
            "b16": b16,
            "deg1": deg1_arr.astype(bf),
            "self8": self8_arr,
            "ef": ef_arr,
            "oh": oh_arr.reshape(128, S * 128),
            "gidx": gidx_full,
        })

    _last_in_maps = in_maps
    res = run_bass_kernel_spmd(nc, in_maps, list(range(N_CORES)))
    out = np.concatenate([res.results[k]["out"] for k in range(N_CORES)], axis=0)
    return out


# revision 19
# speedup vs baseline: 1.0745x; 1.0745x over previous
"""GNN message-passing kernel for 8 Trainium2 NeuronCores.

Reference computation:
    t   = node_feats @ W + b                       # [N, H]
    msgs = t[nbr] + edge_feats[eid]                # [E, H]
    agg = segment_sum(msgs, dst, N)                # [N, H]
    out = t + agg

Sharding: dst is sorted, so core k owns the node range [k*6250, (k+1)*6250)
and the contiguous edge slice whose dst falls in that range. No collectives:
by linearity, sum_e t[nbr_e] = (sum_e nf[nbr_e]) @ W + deg*b, so each core
gathers raw fp8 node features from a replicated table and applies W once per
window after aggregation:
    out = (nf_own + sum nf[nbr]) @ W + (1 + deg) * b + sum ef

Per core, edges are grouped into 49 windows of 128 dst nodes, each split in
two gather groups by neighbor id (A: nbr < 25000, B: rest) so gather indices
fit int16. Slot columns are laid out A-blocks first, then B-blocks, and the
gathers are merged: one dma_gather covers a CHUNK of consecutive windows'
A (or B) columns, cutting the Pool-engine descriptor-generation serial cost
~4x versus per-window calls. Pad slots carry index 0 (a real row, so no NaNs
ever enter SBUF and no memsets or count registers are needed); the one-hot
rows of pad slots are zero, which nulls their contribution.

Each window accumulates:
  psa[f_lo, d] += g[:, cH:cH+128].T @ onehot ; self[:, :128].T @ I   (fp8)
  psb[f_hi, d] += g[:, cH+128:+256].T @ onehot ; self half           (fp8)
  pso[d, h]    += onehot.T @ ef_tile  + (1+deg).T @ b
then the W-transform: copy psa/psb to SBUF bf16 S_T and
  pso += S_T.T @ W  (two K=128 bf16 matmuls), flush pso to HBM.
"""

import sys

sys.path.insert(0, "/opt/trn_rl_repo")

import ml_dtypes
import numpy as np

import concourse.bacc as bacc
import concourse.mybir as mybir
import concourse.tile as tile
from concourse.bass_utils import run_bass_kernel_spmd
from concourse.library_config import mlp

N_NODES = 50000
N_EDGES = 800000
H = 256
N_CORES = 8
NODES_PER_CORE = N_NODES // N_CORES          # 6250
WIN = 128                                    # dst nodes per PSUM window
N_WIN = (NODES_PER_CORE + WIN - 1) // WIN    # 49 (last window = 106 nodes)
SPLIT = 25000                                # A: nbr < SPLIT, B: rest
CHUNK = 2                                    # windows per merged gather call

F8 = mybir.dt.float8e3                       # e3m4: 4 mantissa bits
F8NP = ml_dtypes.float8_e3m4

_cache = {}
_last_in_maps = None


def _build_schedule(dst, nbr):
    """Host-side slot schedule, shared shapes across cores (single NEFF).

    Column space: A-group columns of all windows first (c0[w, 0]), then all
    B-group columns (c0[w, 1]). Returns (T, c0, S, per_core).
    """
    bounds = np.searchsorted(dst, np.arange(N_CORES + 1) * NODES_PER_CORE)
    counts = np.zeros((N_CORES, N_WIN, 2), dtype=np.int64)
    per_core_raw = []
    for k in range(N_CORES):
        e0, e1 = bounds[k], bounds[k + 1]
        dk = dst[e0:e1].astype(np.int64) - k * NODES_PER_CORE
        nk = nbr[e0:e1].astype(np.int64)
        w = dk >> 7
        q = (nk >= SPLIT).astype(np.int64)
        key = q * N_WIN + w
        np.add.at(counts[k], (w, q), 1)
        per_core_raw.append((e0, e1, dk, nk, key))

    T = np.maximum((counts.max(axis=0) + 127) // 128, 1)   # [N_WIN, 2]
    c0 = np.zeros((N_WIN, 2), dtype=np.int64)
    s = 0
    for q in range(2):
        for w in range(N_WIN):
            c0[w, q] = s
            s += T[w, q]
    S = int(s)

    per_core = []
    for k in range(N_CORES):
        e0, e1, dk, nk, key = per_core_raw[k]
        # sort slots by nbr within each group: ascending gather addresses
        order = np.lexsort((nk, key))
        sorted_key = key[order]
        group_start = np.searchsorted(sorted_key, np.arange(N_WIN * 2))
        j_within = np.arange(len(order)) - group_start[sorted_key]
        slot = np.empty(len(order), dtype=np.int64)
        slot[order] = j_within
        base_col = c0[key % N_WIN, key // N_WIN]
        p = slot % 128
        c = base_col + slot // 128
        per_core.append((e0, e1, dk, nk, p, c))
    return T, c0, S, per_core


def _build_program(T, c0, S):
    nc = bacc.Bacc(num_swdge_queues=4, dynamic_dma_scratch_size=36864)
    f32, bf16, i16 = mybir.dt.float32, mybir.dt.bfloat16, mybir.dt.int16

    nf8 = nc.declare_dram_parameter("nf8", [N_NODES, H], F8, isOutput=False)
    identp = nc.declare_dram_parameter("ident", [128, 128], F8, isOutput=False)
    W16 = nc.declare_dram_parameter("W16", [H, H], bf16, isOutput=False)
    b16p = nc.declare_dram_parameter("b16", [1, H], bf16, isOutput=False)
    deg1 = nc.declare_dram_parameter("deg1", [1, N_WIN * WIN], bf16, isOutput=False)
    self8p = nc.declare_dram_parameter("self8", [128, N_WIN * H], F8, isOutput=False)
    ef = nc.declare_dram_parameter("ef", [128, S, H], F8, isOutput=False)
    ohp = nc.declare_dram_parameter("oh", [128, S * 128], F8, isOutput=False)
    gidx = nc.declare_dram_parameter("gidx", [128, 8 * S], i16, isOutput=False)
    outp = nc.declare_dram_parameter("out", [NODES_PER_CORE, H], f32, isOutput=True)

    n_chunks = (N_WIN + CHUNK - 1) // CHUNK
    # columns covered by chunk i, group q: [cc0[i][q], cc1[i][q])
    cc0 = [[int(c0[i * CHUNK, q]) for q in range(2)] for i in range(n_chunks)]
    cc1 = [
        [
            int(
                c0[min((i + 1) * CHUNK, N_WIN) - 1, q]
                + T[min((i + 1) * CHUNK, N_WIN) - 1, q]
            )
            for q in range(2)
        ]
        for i in range(n_chunks)
    ]
    camax = max(cc1[i][0] - cc0[i][0] for i in range(n_chunks))
    cbmax = max(cc1[i][1] - cc0[i][1] for i in range(n_chunks))
    LAG = 6

    with tile.TileContext(nc) as tc:
        nc.gpsimd.load_library(mlp)
        with (
            tc.tile_pool(name="const", bufs=1) as cpool,
            tc.tile_pool(name="psA", bufs=2, space="PSUM") as ppa,
            tc.tile_pool(name="psB", bufs=2, space="PSUM") as ppb,
            tc.tile_pool(name="psO", bufs=3, space="PSUM") as ppo,
            tc.tile_pool(name="gathA", bufs=3) as gpa,
            tc.tile_pool(name="gathB", bufs=3) as gpb,
            tc.tile_pool(name="ef8", bufs=4) as e8p,
            tc.tile_pool(name="oneh", bufs=4) as ohpool,
            tc.tile_pool(name="sT", bufs=3) as stp,
            tc.tile_pool(name="flush", bufs=3) as flp,
        ):
            # resident constants; gidx first so gathers can start early
            gidx_s = cpool.tile([128, 8 * S], i16)
            third = ((8 * S) // 3) & ~7
            nc.sync.dma_start(out=gidx_s[:, :third], in_=gidx[:, :third])
            nc.sync.dma_start(out=gidx_s[:, third:2 * third], in_=gidx[:, third:2 * third])
            nc.sync.dma_start(out=gidx_s[:, 2 * third:], in_=gidx[:, 2 * third:])
            id8 = cpool.tile([128, 128], F8)
            nc.scalar.dma_start(out=id8[:], in_=identp[:])
            w16 = cpool.tile([128, 2 * H], bf16)           # W in two K-halves
            nc.scalar.dma_start(out=w16[:, :H], in_=W16[0:128, :])
            nc.scalar.dma_start(out=w16[:, H:], in_=W16[128:256, :])
            b16 = cpool.tile([1, H], bf16)
            nc.scalar.dma_start(out=b16[:], in_=b16p[:])
            d16 = cpool.tile([1, N_WIN * WIN], bf16)
            nc.scalar.dma_start(out=d16[:], in_=deg1[:])
            self_s = cpool.tile([128, N_WIN * H], F8)
            nc.scalar.dma_start(out=self_s[:], in_=self8p[:])

            g_tiles = {}
            eo_tiles = {}

            def issue_gathers(i):
                for q, (pool, wmax) in enumerate(((gpa, camax), (gpb, cbmax))):
                    lo, hi = cc0[i][q], cc1[i][q]
                    ncols = hi - lo
                    g = pool.tile([128, wmax * H], F8, tag=f"g{q}")
                    g_tiles[(i, q)] = g
                    src = nf8[0:32768, :] if q == 0 else nf8[SPLIT:N_NODES, :]
                    nc.gpsimd.dma_gather(
                        out_ap=g[:, :ncols * H].rearrange("p (c d) -> p c d", d=H),
                        in_ap=src,
                        idxs_ap=gidx_s[:, 8 * lo: 8 * hi],
                        num_idxs=ncols * 128,
                        num_idxs_reg=ncols * 128,
                        elem_size=H,
                        single_packet=False,
                        queue_num=(2 * i + q) % 4,
                    )

            def issue_streams(w):
                res = []
                for q in range(2):
                    tw = int(T[w, q])
                    cc = int(c0[w, q])
                    e8 = e8p.tile([128, tw * H], F8, tag=f"ef{q}")
                    nc.sync.dma_start(
                        out=e8[:].rearrange("p (c d) -> p c d", d=H),
                        in_=ef[:, cc:cc + tw, :],
                    )
                    oh = ohpool.tile([128, tw * 128], F8, tag=f"oh{q}")
                    nc.scalar.dma_start(out=oh[:], in_=ohp[:, cc * 128:(cc + tw) * 128])
                    res.append((e8, oh))
                eo_tiles[w] = res

            pend = {}

            def compute_ef(w):
                n0 = w * WIN
                psa = ppa.tile([128, 128], f32, tag="psa")
                psb = ppb.tile([128, 128], f32, tag="psb")
                pso = ppo.tile([128, H], f32, tag="pso")
                eo = eo_tiles.pop(w)
                # self term opens the psT groups; bias opens pso
                sl = self_s[:, w * H:(w + 1) * H]
                nc.tensor.matmul(psa[:], lhsT=sl[:, :128], rhs=id8[:], start=True, stop=False)
                nc.tensor.matmul(psb[:], lhsT=sl[:, 128:], rhs=id8[:], start=True, stop=False)
                nc.tensor.matmul(pso[:], lhsT=d16[:, n0:n0 + WIN], rhs=b16[:], start=True, stop=False)
                for q in range(2):
                    tw = int(T[w, q])
                    e8, oh = eo[q]
                    for c in range(tw):
                        ohc = oh[:, c * 128:(c + 1) * 128]
                        nc.tensor.matmul(pso[:], lhsT=ohc, rhs=e8[:, c * H:(c + 1) * H],
                                         start=False, stop=False)
                pend[w] = (psa, psb, pso, eo)

            def compute_g(w):
                psa, psb, pso, eo = pend[w]
                i = w // CHUNK
                for q in range(2):
                    tw = int(T[w, q])
                    _, oh = eo[q]
                    g = g_tiles[(i, q)]
                    goff = int(c0[w, q]) - cc0[i][q]
                    for c in range(tw):
                        ohc = oh[:, c * 128:(c + 1) * 128]
                        gc = g[:, (goff + c) * H:(goff + c + 1) * H]
                        last = q == 1 and c == tw - 1
                        nc.tensor.matmul(psa[:], lhsT=gc[:, :128], rhs=ohc,
                                         start=False, stop=last)
                        nc.tensor.matmul(psb[:], lhsT=gc[:, 128:], rhs=ohc,
                                         start=False, stop=last)

            def transform(w):
                psa, psb, pso, _ = pend.pop(w)
                n0 = w * WIN
                nn = min(WIN, NODES_PER_CORE - n0)
                st = stp.tile([128, H], bf16, tag="sT")
                nc.vector.tensor_copy(out=st[:, :128], in_=psa[:])
                nc.vector.tensor_copy(out=st[:, 128:], in_=psb[:])
                nc.tensor.matmul(pso[:], lhsT=st[:, :128], rhs=w16[:, :H],
                                 start=False, stop=False)
                nc.tensor.matmul(pso[:], lhsT=st[:, 128:], rhs=w16[:, H:],
                                 start=False, stop=True)
                fl = flp.tile([128, H], f32, tag="flush")
                nc.scalar.copy(out=fl[:nn, :], in_=pso[:nn, :])
                nc.scalar.dma_start(out=outp[n0:n0 + nn, :], in_=fl[:nn, :])

            for step in range(N_WIN + LAG):
                if step < N_WIN and step % CHUNK == 0:
                    issue_gathers(step // CHUNK)
                ws = step - (LAG - 3)
                if 0 <= ws < N_WIN:
                    issue_streams(ws)
                if step >= LAG:
                    w = step - LAG
                    compute_ef(w)
                    if w > 0:
                        transform(w - 1)
                    compute_g(w)
            transform(N_WIN - 1)

    nc.compile()
    return nc


def kernel(node_feats, edge_feats, W, b, dst, nbr, eid):
    global _last_in_maps
    node_feats = np.ascontiguousarray(np.asarray(node_feats, dtype=np.float32))
    edge_feats = np.ascontiguousarray(np.asarray(edge_feats, dtype=np.float32))
    W = np.ascontiguousarray(np.asarray(W, dtype=np.float32))
    b = np.asarray(b, dtype=np.float32).reshape(1, H)
    dst = np.asarray(dst, dtype=np.int32)
    nbr = np.asarray(nbr, dtype=np.int32)
    eid = np.asarray(eid, dtype=np.int32)

    T, c0, S, per_core = _build_schedule(dst, nbr)

    key = (S, T.tobytes())
    if key not in _cache:
        _cache.clear()
        _cache[key] = _build_program(T, c0, S)
    nc = _cache[key]

    bf = ml_dtypes.bfloat16
    nf8_arr = node_feats.astype(F8NP)
    ident = np.zeros((128, 128), dtype=F8NP)
    np.fill_diagonal(ident, F8NP(1.0))
    W16 = W.astype(bf)
    b16 = b.astype(bf)

    n_chunks = (N_WIN + CHUNK - 1) // CHUNK
    # column -> gather-call base column (for relative j encoding)
    callbase = np.zeros(S, dtype=np.int64)
    for i in range(n_chunks):
        w0 = i * CHUNK
        w1 = min(w0 + CHUNK, N_WIN)
        for q in range(2):
            lo = c0[w0, q]
            hi = c0[w1 - 1, q] + T[w1 - 1, q]
            callbase[lo:hi] = lo

    in_maps = []
    for k in range(N_CORES):
        e0, e1, dk, nk, p, c = per_core[k]
        ef_arr = np.zeros((128, S, H), dtype=F8NP)
        ef_arr[p, c] = edge_feats[eid[e0:e1]].astype(F8NP)
        oh_arr = np.zeros((128, S, 128), dtype=F8NP)
        oh_arr[p, c, dk & 127] = F8NP(1.0)
        # gather indices: call-relative slot j = (c - callbase)*128 + p,
        # encoded at [j%16, 8*callbase + j//16]; pad slots keep index 0
        gidx_arr = np.zeros((16, 8 * S), dtype=np.int16)
        q_arr = (nk >= SPLIT).astype(np.int64)
        idx_val = np.where(q_arr == 0, nk, nk - SPLIT)
        base_col = callbase[c]
        j = (c - base_col) * 128 + p
        gidx_arr[j % 16, 8 * base_col + j // 16] = idx_val.astype(np.int16)
        gidx_full = np.tile(gidx_arr, (8, 1))
        w_arr = dk >> 7
        deg1_arr = np.zeros((1, N_WIN * WIN), dtype=np.float32)
        deg1_arr[0, :NODES_PER_CORE] = 1.0
        np.add.at(deg1_arr[0], dk, 1.0)
        own8 = np.zeros((N_WIN * WIN, H), dtype=F8NP)
        own8[:NODES_PER_CORE] = nf8_arr[k * NODES_PER_CORE:(k + 1) * NODES_PER_CORE]
        self8_arr = np.ascontiguousarray(
            own8.reshape(N_WIN, WIN, H).transpose(1, 0, 2).reshape(WIN, N_WIN * H)
        )
        in_maps.append({
            "nf8": nf8_arr,
            "ident": ident,
            "W16": W16,
            "b16": b16,
            "deg1": deg1_arr.astype(bf),
            "self8": self8_arr,
            "ef": ef_arr,
            "oh": oh_arr.reshape(128, S * 128),
            "gidx": gidx_full,
        })

    _last_in_maps = in_maps
    res = run_bass_kernel_spmd(nc, in_maps, list(range(N_CORES)))
    out = np.concatenate([res.results[k]["out"] for k in range(N_CORES)], axis=0)
    return out


# revision 20
# speedup vs baseline: 1.2342x; 1.1487x over previous
"""GNN message-passing kernel for 8 Trainium2 NeuronCores.

Reference computation:
    t   = node_feats @ W + b                       # [N, H]
    msgs = t[nbr] + edge_feats[eid]                # [E, H]
    agg = segment_sum(msgs, dst, N)                # [N, H]
    out = t + agg

Sharding: dst is sorted, so core k owns the node range [k*6250, (k+1)*6250)
and the contiguous edge slice whose dst falls in that range. No collectives:
by linearity, sum_e t[nbr_e] = (sum_e nf[nbr_e]) @ W + deg*b, so each core
gathers raw fp8 node features from a replicated table and applies W once per
window after aggregation:
    out = (nf_own + sum nf[nbr]) @ W + (1 + deg) * b + sum ef

Per core, edges are grouped into 49 windows of 128 dst nodes, each split in
two gather groups by neighbor id (A: nbr < 25000, B: rest) so gather indices
fit int16. Slots within a group are nbr-sorted and dense; trailing pad slots
carry index -1 and the per-core count register trims them.

Each window accumulates two PSUM tiles:
  psT[f, d]  (two 128-row halves) += g_tile[:, f_half].T @ onehot    (fp8)
                                   += self_tile[:, f_half].T @ I     (fp8)
  pso[d, h]  += onehot.T @ ef_tile                                   (fp8)
             += (1+deg).T @ b                                        (K=1)
then the W-transform: copy psT to SBUF bf16 and
  pso += S.T @ W  (two K=128 bf16 matmuls), flush pso to HBM.

The one-hot (pad rows all-zero, so garbage in count-trimmed gather slots is
multiplied by zero; the gather pool is also zeroed once so no NaNs appear),
edge features and gather indices are host-built and streamed; all 8-bit
tensors are e3m4.
"""

import sys

sys.path.insert(0, "/opt/trn_rl_repo")

import ml_dtypes
import numpy as np

import concourse.bacc as bacc
import concourse.mybir as mybir
import concourse.tile as tile
from concourse.bass_utils import run_bass_kernel_spmd
from concourse.library_config import mlp

N_NODES = 50000
N_EDGES = 800000
H = 256
N_CORES = 8
NODES_PER_CORE = N_NODES // N_CORES          # 6250
WIN = 128                                    # dst nodes per PSUM window
N_WIN = (NODES_PER_CORE + WIN - 1) // WIN    # 49 (last window = 106 nodes)
SPLIT = 25000                                # A: nbr < SPLIT, B: rest

F8 = mybir.dt.float8e3                       # e3m4: 4 mantissa bits
F8NP = ml_dtypes.float8_e3m4

_cache = {}
_last_in_maps = None


def _build_schedule(dst, nbr):
    """Host-side slot schedule, shared shapes across cores (single NEFF).

    Returns (T, c0, S, per_core): T[(w, q)] tile count of window w group q,
    c0[(w, q)] its starting slot-column, S total slot-columns, per_core[k]
    the per-core edge->slot assignment.
    """
    bounds = np.searchsorted(dst, np.arange(N_CORES + 1) * NODES_PER_CORE)
    counts = np.zeros((N_CORES, N_WIN, 2), dtype=np.int64)
    per_core_raw = []
    for k in range(N_CORES):
        e0, e1 = bounds[k], bounds[k + 1]
        dk = dst[e0:e1].astype(np.int64) - k * NODES_PER_CORE
        nk = nbr[e0:e1].astype(np.int64)
        w = dk >> 7
        q = (nk >= SPLIT).astype(np.int64)
        key = w * 2 + q
        np.add.at(counts[k], (w, q), 1)
        per_core_raw.append((e0, e1, dk, nk, key))

    T = np.maximum((counts.max(axis=0) + 127) // 128, 1)   # [N_WIN, 2]
    c0 = np.zeros((N_WIN, 2), dtype=np.int64)
    s = 0
    for w in range(N_WIN):
        for q in range(2):
            c0[w, q] = s
            s += T[w, q]
    S = int(s)

    per_core = []
    for k in range(N_CORES):
        e0, e1, dk, nk, key = per_core_raw[k]
        # sort slots by nbr within each group: ascending gather addresses
        order = np.lexsort((nk, key))
        sorted_key = key[order]
        group_start = np.searchsorted(sorted_key, np.arange(N_WIN * 2))
        j_within = np.arange(len(order)) - group_start[sorted_key]
        slot = np.empty(len(order), dtype=np.int64)
        slot[order] = j_within
        base_col = c0[key >> 1, key & 1]
        p = slot % 128
        c = base_col + slot // 128
        per_core.append((e0, e1, dk, nk, p, c))
    return T, c0, S, per_core


def _build_program(T, c0, S):
    nc = bacc.Bacc(num_swdge_queues=4, dynamic_dma_scratch_size=36864)
    f32, bf16, i16 = mybir.dt.float32, mybir.dt.bfloat16, mybir.dt.int16
    i32 = mybir.dt.int32

    nf8 = nc.declare_dram_parameter("nf8", [N_NODES, H], F8, isOutput=False)
    identp = nc.declare_dram_parameter("ident", [128, 128], F8, isOutput=False)
    W16 = nc.declare_dram_parameter("W16", [H, H], bf16, isOutput=False)
    b16p = nc.declare_dram_parameter("b16", [1, H], bf16, isOutput=False)
    deg1 = nc.declare_dram_parameter("deg1", [1, N_WIN * WIN], bf16, isOutput=False)
    self8p = nc.declare_dram_parameter("self8", [128, N_WIN * H], F8, isOutput=False)
    ef = nc.declare_dram_parameter("ef", [128, S, H], F8, isOutput=False)
    ohp = nc.declare_dram_parameter("oh", [128, S * 128], F8, isOutput=False)
    cntp = nc.declare_dram_parameter("cnt", [1, 2 * N_WIN], i32, isOutput=False)
    gidx = nc.declare_dram_parameter("gidx", [128, 8 * S], i16, isOutput=False)
    outp = nc.declare_dram_parameter("out", [NODES_PER_CORE, H], f32, isOutput=True)

    twmax = int(T.max())
    LAG = 3

    with tile.TileContext(nc) as tc:
        nc.gpsimd.load_library(mlp)
        with (
            tc.tile_pool(name="const", bufs=1) as cpool,
            tc.tile_pool(name="psA", bufs=2, space="PSUM") as ppa,
            tc.tile_pool(name="psB", bufs=2, space="PSUM") as ppb,
            tc.tile_pool(name="psO", bufs=3, space="PSUM") as ppo,
            tc.tile_pool(name="gath", bufs=2 * LAG + 4) as gp,
            tc.tile_pool(name="ef8", bufs=LAG + 2) as e8p,
            tc.tile_pool(name="oneh", bufs=LAG + 2) as ohpool,
            tc.tile_pool(name="sT", bufs=3) as stp,
            tc.tile_pool(name="flush", bufs=3) as flp,
        ):
            # resident constants; gidx/cnt first so gathers can start early
            cnt_s = cpool.tile([1, 2 * N_WIN], i32)
            nc.scalar.dma_start(out=cnt_s[:], in_=cntp[:])
            gidx_s = cpool.tile([128, 8 * S], i16)
            third = ((8 * S) // 3) & ~7
            nc.sync.dma_start(out=gidx_s[:, :third], in_=gidx[:, :third])
            nc.sync.dma_start(out=gidx_s[:, third:2 * third], in_=gidx[:, third:2 * third])
            nc.sync.dma_start(out=gidx_s[:, 2 * third:], in_=gidx[:, 2 * third:])
            id8 = cpool.tile([128, 128], F8)
            nc.scalar.dma_start(out=id8[:], in_=identp[:])
            w16 = cpool.tile([128, 2 * H], bf16)           # W in two K-halves
            nc.scalar.dma_start(out=w16[:, :H], in_=W16[0:128, :])
            nc.scalar.dma_start(out=w16[:, H:], in_=W16[128:256, :])
            b16 = cpool.tile([1, H], bf16)
            nc.scalar.dma_start(out=b16[:], in_=b16p[:])
            d16 = cpool.tile([1, N_WIN * WIN], bf16)
            nc.scalar.dma_start(out=d16[:], in_=deg1[:])
            self_s = cpool.tile([128, N_WIN * H], F8)
            nc.scalar.dma_start(out=self_s[:], in_=self8p[:])

            # zero the gather pool once: count-trimmed pad slots must hold
            # finite fp8 values (the one-hot zero rows null them out, but
            # NaN * 0 would still poison the psum)
            for _ in range(2 * LAG + 4):
                z = gp.tile([128, twmax * H], F8, tag="gath")
                nc.vector.memset(z[:], 0.0)

            creg = nc.gpsimd.alloc_register("gather_cnt")
            g_tiles = {}
            eo_tiles = {}

            def issue_dmas(w):
                for q in range(2):
                    tw = int(T[w, q])
                    cc = int(c0[w, q])
                    g = gp.tile([128, twmax * H], F8, tag="gath")
                    g_tiles[(w, q)] = g
                    src = nf8[0:32768, :] if q == 0 else nf8[SPLIT:N_NODES, :]
                    nc.gpsimd.reg_load(creg, cnt_s[0:1, 2 * w + q:2 * w + q + 1])
                    nc.gpsimd.dma_gather(
                        out_ap=g[:, :tw * H].rearrange("p (c d) -> p c d", d=H),
                        in_ap=src,
                        idxs_ap=gidx_s[:, 8 * cc: 8 * (cc + tw)],
                        num_idxs=tw * 128,
                        num_idxs_reg=creg,
                        elem_size=H,
                        single_packet=False,
                        queue_num=(2 * w + q) % 4,
                    )
                twin = int(T[w, 0] + T[w, 1])
                cc = int(c0[w, 0])
                e8 = e8p.tile([128, twin * H], F8, tag="ef8")
                nc.sync.dma_start(
                    out=e8[:].rearrange("p (c d) -> p c d", d=H),
                    in_=ef[:, cc:cc + twin, :],
                )
                oh = ohpool.tile([128, twin * 128], F8, tag="oneh")
                nc.sync.dma_start(out=oh[:], in_=ohp[:, cc * 128:(cc + twin) * 128])
                eo_tiles[w] = (e8, oh)

            pend = {}

            def compute_ef(w):
                n0 = w * WIN
                psa = ppa.tile([128, 128], f32, tag="psa")
                psb = ppb.tile([128, 128], f32, tag="psb")
                pso = ppo.tile([128, H], f32, tag="pso")
                e8, oh = eo_tiles.pop(w)
                # self term opens the psT groups; bias opens pso
                sl = self_s[:, w * H:(w + 1) * H]
                nc.tensor.matmul(psa[:], lhsT=sl[:, :128], rhs=id8[:], start=True, stop=False)
                nc.tensor.matmul(psb[:], lhsT=sl[:, 128:], rhs=id8[:], start=True, stop=False)
                nc.tensor.matmul(pso[:], lhsT=d16[:, n0:n0 + WIN], rhs=b16[:], start=True, stop=False)
                # edge features (ready early)
                twin = int(T[w, 0] + T[w, 1])
                for c in range(twin):
                    ohc = oh[:, c * 128:(c + 1) * 128]
                    nc.tensor.matmul(pso[:], lhsT=ohc, rhs=e8[:, c * H:(c + 1) * H],
                                     start=False, stop=False)
                pend[w] = (psa, psb, pso, oh)

            def compute_g(w):
                psa, psb, pso, oh = pend[w]
                for q in range(2):
                    tw = int(T[w, q])
                    off = 0 if q == 0 else int(T[w, 0])
                    g = g_tiles.pop((w, q))
                    for c in range(tw):
                        ohc = oh[:, (off + c) * 128:(off + c + 1) * 128]
                        gc = g[:, c * H:(c + 1) * H]
                        last = q == 1 and c == tw - 1
                        nc.tensor.matmul(psa[:], lhsT=gc[:, :128], rhs=ohc,
                                         start=False, stop=last)
                        nc.tensor.matmul(psb[:], lhsT=gc[:, 128:], rhs=ohc,
                                         start=False, stop=last)

            def transform(w):
                psa, psb, pso, _ = pend.pop(w)
                n0 = w * WIN
                nn = min(WIN, NODES_PER_CORE - n0)
                st = stp.tile([128, H], bf16, tag="sT")
                nc.vector.tensor_copy(out=st[:, :128], in_=psa[:])
                nc.vector.tensor_copy(out=st[:, 128:], in_=psb[:])
                nc.tensor.matmul(pso[:], lhsT=st[:, :128], rhs=w16[:, :H],
                                 start=False, stop=False)
                nc.tensor.matmul(pso[:], lhsT=st[:, 128:], rhs=w16[:, H:],
                                 start=False, stop=True)
                fl = flp.tile([128, H], f32, tag="flush")
                nc.scalar.copy(out=fl[:nn, :], in_=pso[:nn, :])
                nc.scalar.dma_start(out=outp[n0:n0 + nn, :], in_=fl[:nn, :])

            for step in range(N_WIN + LAG):
                if step < N_WIN:
                    issue_dmas(step)
                if step >= LAG:
                    w = step - LAG
                    compute_ef(w)
                    if w > 0:
                        transform(w - 1)
                    compute_g(w)
            transform(N_WIN - 1)

    nc.compile()
    return nc


def kernel(node_feats, edge_feats, W, b, dst, nbr, eid):
    global _last_in_maps
    node_feats = np.ascontiguousarray(np.asarray(node_feats, dtype=np.float32))
    edge_feats = np.ascontiguousarray(np.asarray(edge_feats, dtype=np.float32))
    W = np.ascontiguousarray(np.asarray(W, dtype=np.float32))
    b = np.asarray(b, dtype=np.float32).reshape(1, H)
    dst = np.asarray(dst, dtype=np.int32)
    nbr = np.asarray(nbr, dtype=np.int32)
    eid = np.asarray(eid, dtype=np.int32)

    T, c0, S, per_core = _build_schedule(dst, nbr)

    key = (S, T.tobytes())
    if key not in _cache:
        _cache.clear()
        _cache[key] = _build_program(T, c0, S)
    nc = _cache[key]

    bf = ml_dtypes.bfloat16
    nf8_arr = node_feats.astype(F8NP)
    ident = np.zeros((128, 128), dtype=F8NP)
    np.fill_diagonal(ident, F8NP(1.0))
    W16 = W.astype(bf)
    b16 = b.astype(bf)

    in_maps = []
    for k in range(N_CORES):
        e0, e1, dk, nk, p, c = per_core[k]
        ef_arr = np.zeros((128, S, H), dtype=F8NP)
        ef_arr[p, c] = edge_feats[eid[e0:e1]].astype(F8NP)
        oh_arr = np.zeros((128, S, 128), dtype=F8NP)
        oh_arr[p, c, dk & 127] = F8NP(1.0)
        # gather indices: slot-within-group j = (c - group_base)*128 + p,
        # encoded at [j%16, 8*group_base + j//16]; unused trailing slots
        # stay -1 so the count-register gather skips them
        gidx_arr = np.full((16, 8 * S), -1, dtype=np.int16)
        w_arr = dk >> 7
        q_arr = (nk >= SPLIT).astype(np.int64)
        idx_val = np.where(q_arr == 0, nk, nk - SPLIT)
        base_col = c0[w_arr, q_arr]
        j = (c - base_col) * 128 + p
        gidx_arr[j % 16, 8 * base_col + j // 16] = idx_val.astype(np.int16)
        gidx_full = np.tile(gidx_arr, (8, 1))
        cnt_arr = np.zeros((1, 2 * N_WIN), dtype=np.int32)
        np.add.at(cnt_arr[0], w_arr * 2 + q_arr, 1)
        deg1_arr = np.zeros((1, N_WIN * WIN), dtype=np.float32)
        deg1_arr[0, :NODES_PER_CORE] = 1.0
        np.add.at(deg1_arr[0], dk, 1.0)
        # own-node fp8 rows: self8[p, w*H + f] = nf8 row of node w*128+p
        own8 = np.zeros((N_WIN * WIN, H), dtype=F8NP)
        own8[:NODES_PER_CORE] = nf8_arr[k * NODES_PER_CORE:(k + 1) * NODES_PER_CORE]
        self8_arr = np.ascontiguousarray(
            own8.reshape(N_WIN, WIN, H).transpose(1, 0, 2).reshape(WIN, N_WIN * H)
        )
        in_maps.append({
            "nf8": nf8_arr,
            "ident": ident,
            "W16": W16,
            "b16": b16,
            "deg1": deg1_arr.astype(bf),
            "self8": self8_arr,
            "ef": ef_arr,
            "oh": oh_arr.reshape(128, S * 128),
            "cnt": cnt_arr,
            "gidx": gidx_full,
        })

    _last_in_maps = in_maps
    res = run_bass_kernel_spmd(nc, in_maps, list(range(N_CORES)))
    out = np.concatenate([res.results[k]["out"] for k in range(N_CORES)], axis=0)
    return out


# revision 22
# speedup vs baseline: 1.2652x; 1.0251x over previous
"""GNN message-passing kernel for 8 Trainium2 NeuronCores.

Reference computation:
    t   = node_feats @ W + b                       # [N, H]
    msgs = t[nbr] + edge_feats[eid]                # [E, H]
    agg = segment_sum(msgs, dst, N)                # [N, H]
    out = t + agg

Sharding: dst is sorted, so core k owns the node range [k*6250, (k+1)*6250)
and the contiguous edge slice whose dst falls in that range. No collectives:
by linearity, sum_e t[nbr_e] = (sum_e nf[nbr_e]) @ W + deg*b, so each core
gathers raw fp8 node features from a replicated table and applies W once per
window after aggregation:
    out = (nf_own + sum nf[nbr]) @ W + (1 + deg) * b + sum ef

Per core, edges are grouped into 49 windows of 128 dst nodes, each split in
two gather groups by neighbor id (A: nbr < 25000, B: rest) so gather indices
fit int16. Slots within a group are nbr-sorted and dense; trailing pad slots
carry index -1 and the per-core count register trims them.

Each window accumulates two PSUM tiles:
  psT[f, d]  (two 128-row halves) += g_tile[:, f_half].T @ onehot    (fp8)
                                   += self_tile[:, f_half].T @ I     (fp8)
  pso[d, h]  += onehot.T @ ef_tile                                   (fp8)
             += (1+deg).T @ b                                        (K=1)
then the W-transform: copy psT to SBUF bf16 and
  pso += S.T @ W  (two K=128 bf16 matmuls), flush pso to HBM.

The one-hot (pad rows all-zero, so garbage in count-trimmed gather slots is
multiplied by zero; the gather pool is also zeroed once so no NaNs appear),
edge features and gather indices are host-built and streamed; all 8-bit
tensors are e3m4.
"""

import sys

sys.path.insert(0, "/opt/trn_rl_repo")

import ml_dtypes
import numpy as np

import concourse.bacc as bacc
import concourse.mybir as mybir
import concourse.tile as tile
from concourse.bass_utils import run_bass_kernel_spmd
from concourse.library_config import mlp

N_NODES = 50000
N_EDGES = 800000
H = 256
N_CORES = 8
NODES_PER_CORE = N_NODES // N_CORES          # 6250
WIN = 128                                    # dst nodes per PSUM window
N_WIN = (NODES_PER_CORE + WIN - 1) // WIN    # 49 (last window = 106 nodes)
SPLIT = 25000                                # A: nbr < SPLIT, B: rest

F8 = mybir.dt.float8e3                       # e3m4: 4 mantissa bits
F8NP = ml_dtypes.float8_e3m4

_cache = {}
_last_in_maps = None


def _build_schedule(dst, nbr):
    """Host-side slot schedule, shared shapes across cores (single NEFF).

    Returns (T, c0, S, per_core): T[(w, q)] tile count of window w group q,
    c0[(w, q)] its starting slot-column, S total slot-columns, per_core[k]
    the per-core edge->slot assignment.
    """
    bounds = np.searchsorted(dst, np.arange(N_CORES + 1) * NODES_PER_CORE)
    counts = np.zeros((N_CORES, N_WIN, 2), dtype=np.int64)
    per_core_raw = []
    for k in range(N_CORES):
        e0, e1 = bounds[k], bounds[k + 1]
        dk = dst[e0:e1].astype(np.int64) - k * NODES_PER_CORE
        nk = nbr[e0:e1].astype(np.int64)
        w = dk >> 7
        q = (nk >= SPLIT).astype(np.int64)
        key = w * 2 + q
        np.add.at(counts[k], (w, q), 1)
        per_core_raw.append((e0, e1, dk, nk, key))

    T = np.maximum((counts.max(axis=0) + 127) // 128, 1)   # [N_WIN, 2]
    c0 = np.zeros((N_WIN, 2), dtype=np.int64)
    s = 0
    for w in range(N_WIN):
        for q in range(2):
            c0[w, q] = s
            s += T[w, q]
    S = int(s)

    per_core = []
    for k in range(N_CORES):
        e0, e1, dk, nk, key = per_core_raw[k]
        # sort slots by nbr within each group: ascending gather addresses
        order = np.lexsort((nk, key))
        sorted_key = key[order]
        group_start = np.searchsorted(sorted_key, np.arange(N_WIN * 2))
        j_within = np.arange(len(order)) - group_start[sorted_key]
        slot = np.empty(len(order), dtype=np.int64)
        slot[order] = j_within
        base_col = c0[key >> 1, key & 1]
        p = slot % 128
        c = base_col + slot // 128
        per_core.append((e0, e1, dk, nk, p, c))
    return T, c0, S, per_core


def _build_program(T, c0, S):
    nc = bacc.Bacc(num_swdge_queues=4, dynamic_dma_scratch_size=36864)
    f32, bf16, i16 = mybir.dt.float32, mybir.dt.bfloat16, mybir.dt.int16
    i32 = mybir.dt.int32

    nf8 = nc.declare_dram_parameter("nf8", [N_NODES, H], F8, isOutput=False)
    identp = nc.declare_dram_parameter("ident", [128, 128], F8, isOutput=False)
    W16 = nc.declare_dram_parameter("W16", [H, H], bf16, isOutput=False)
    b16p = nc.declare_dram_parameter("b16", [1, H], bf16, isOutput=False)
    deg1 = nc.declare_dram_parameter("deg1", [1, N_WIN * WIN], bf16, isOutput=False)
    self8p = nc.declare_dram_parameter("self8", [128, N_WIN * H], F8, isOutput=False)
    ef = nc.declare_dram_parameter("ef", [128, S, H], F8, isOutput=False)
    ohp = nc.declare_dram_parameter("oh", [128, S * 128], F8, isOutput=False)
    cntp = nc.declare_dram_parameter("cnt", [1, 2 * N_WIN], i32, isOutput=False)
    gidx = nc.declare_dram_parameter("gidx", [128, 8 * S], i16, isOutput=False)
    outp = nc.declare_dram_parameter("out", [NODES_PER_CORE, H], f32, isOutput=True)

    twmax = int(T.max())
    tpmax = max(
        int(T[w, 0] + T[w, 1] + (T[w + 1, 0] + T[w + 1, 1] if w + 1 < N_WIN else 0))
        for w in range(0, N_WIN, 2)
    )
    LAG = 4

    with tile.TileContext(nc) as tc:
        nc.gpsimd.load_library(mlp)
        with (
            tc.tile_pool(name="const", bufs=1) as cpool,
            tc.tile_pool(name="psA", bufs=2, space="PSUM") as ppa,
            tc.tile_pool(name="psB", bufs=2, space="PSUM") as ppb,
            tc.tile_pool(name="psO", bufs=3, space="PSUM") as ppo,
            tc.tile_pool(name="gath", bufs=2 * LAG + 4) as gp,
            tc.tile_pool(name="ef8", bufs=3) as e8p,
            tc.tile_pool(name="oneh", bufs=3) as ohpool,
            tc.tile_pool(name="sT", bufs=3) as stp,
            tc.tile_pool(name="flush", bufs=3) as flp,
        ):
            # resident constants; gidx/cnt first so gathers can start early
            cnt_s = cpool.tile([1, 2 * N_WIN], i32)
            nc.scalar.dma_start(out=cnt_s[:], in_=cntp[:])
            gidx_s = cpool.tile([128, 8 * S], i16)
            first = 8 * int(c0[6, 0])           # columns of windows 0-5
            half = (first + (8 * S - first) // 2) & ~7
            nc.sync.dma_start(out=gidx_s[:, :first], in_=gidx[:, :first])
            nc.sync.dma_start(out=gidx_s[:, first:half], in_=gidx[:, first:half])
            nc.sync.dma_start(out=gidx_s[:, half:], in_=gidx[:, half:])
            id8 = cpool.tile([128, 128], F8)
            nc.scalar.dma_start(out=id8[:], in_=identp[:])
            w16 = cpool.tile([128, 2 * H], bf16)           # W in two K-halves
            nc.scalar.dma_start(out=w16[:, :H], in_=W16[0:128, :])
            nc.scalar.dma_start(out=w16[:, H:], in_=W16[128:256, :])
            b16 = cpool.tile([1, H], bf16)
            nc.scalar.dma_start(out=b16[:], in_=b16p[:])
            d16 = cpool.tile([1, N_WIN * WIN], bf16)
            nc.scalar.dma_start(out=d16[:], in_=deg1[:])
            self_s = cpool.tile([128, N_WIN * H], F8)
            nc.scalar.dma_start(out=self_s[:], in_=self8p[:])

            # zero the gather pool once: count-trimmed pad slots must hold
            # finite fp8 values (the one-hot zero rows null them out, but
            # NaN * 0 would still poison the psum)
            for _ in range(2 * LAG + 4):
                z = gp.tile([128, twmax * H], F8, tag="gath")
                nc.vector.memset(z[:], 0.0)

            creg = nc.gpsimd.alloc_register("gather_cnt")
            g_tiles = {}
            eo_tiles = {}

            def issue_dmas(w):
                for q in range(2):
                    tw = int(T[w, q])
                    cc = int(c0[w, q])
                    g = gp.tile([128, twmax * H], F8, tag="gath")
                    g_tiles[(w, q)] = g
                    src = nf8[0:32768, :] if q == 0 else nf8[SPLIT:N_NODES, :]
                    nc.gpsimd.reg_load(creg, cnt_s[0:1, 2 * w + q:2 * w + q + 1])
                    nc.gpsimd.dma_gather(
                        out_ap=g[:, :tw * H].rearrange("p (c d) -> p c d", d=H),
                        in_ap=src,
                        idxs_ap=gidx_s[:, 8 * cc: 8 * (cc + tw)],
                        num_idxs=tw * 128,
                        num_idxs_reg=creg,
                        elem_size=H,
                        single_packet=False,
                        queue_num=(2 * w + q) % 4,
                    )
                # ef/oh streamed per 2-window pair: bigger packets, fewer DMAs
                if w % 2 == 0:
                    w1 = min(w + 1, N_WIN - 1)
                    cc = int(c0[w, 0])
                    tpair = int(c0[w1, 0] + T[w1, 0] + T[w1, 1]) - cc
                    e8 = e8p.tile([128, tpmax * H], F8, tag="ef8")
                    nc.sync.dma_start(
                        out=e8[:, :tpair * H].rearrange("p (c d) -> p c d", d=H),
                        in_=ef[:, cc:cc + tpair, :],
                    )
                    oh = ohpool.tile([128, tpmax * 128], F8, tag="oneh")
                    nc.sync.dma_start(out=oh[:, :tpair * 128],
                                      in_=ohp[:, cc * 128:(cc + tpair) * 128])
                    eo_tiles[w] = (e8, oh, 0)
                    if w + 1 < N_WIN:
                        eo_tiles[w + 1] = (e8, oh, int(T[w, 0] + T[w, 1]))

            pend = {}

            def compute_ef(w):
                n0 = w * WIN
                psa = ppa.tile([128, 128], f32, tag="psa")
                psb = ppb.tile([128, 128], f32, tag="psb")
                pso = ppo.tile([128, H], f32, tag="pso")
                e8, oh, coff = eo_tiles.pop(w)
                # self term opens the psT groups; bias opens pso
                sl = self_s[:, w * H:(w + 1) * H]
                nc.tensor.matmul(psa[:], lhsT=sl[:, :128], rhs=id8[:], start=True, stop=False)
                nc.tensor.matmul(psb[:], lhsT=sl[:, 128:], rhs=id8[:], start=True, stop=False)
                nc.tensor.matmul(pso[:], lhsT=d16[:, n0:n0 + WIN], rhs=b16[:], start=True, stop=False)
                # edge features (ready early)
                twin = int(T[w, 0] + T[w, 1])
                for c in range(coff, coff + twin):
                    ohc = oh[:, c * 128:(c + 1) * 128]
                    nc.tensor.matmul(pso[:], lhsT=ohc, rhs=e8[:, c * H:(c + 1) * H],
                                     start=False, stop=False)
                pend[w] = (psa, psb, pso, oh, coff)

            def compute_g(w):
                psa, psb, pso, oh, coff = pend[w]
                for q in range(2):
                    tw = int(T[w, q])
                    off = coff + (0 if q == 0 else int(T[w, 0]))
                    g = g_tiles.pop((w, q))
                    for c in range(tw):
                        ohc = oh[:, (off + c) * 128:(off + c + 1) * 128]
                        gc = g[:, c * H:(c + 1) * H]
                        last = q == 1 and c == tw - 1
                        nc.tensor.matmul(psa[:], lhsT=gc[:, :128], rhs=ohc,
                                         start=False, stop=last)
                        nc.tensor.matmul(psb[:], lhsT=gc[:, 128:], rhs=ohc,
                                         start=False, stop=last)

            def transform(w):
                psa, psb, pso, _, _ = pend.pop(w)
                n0 = w * WIN
                nn = min(WIN, NODES_PER_CORE - n0)
                st = stp.tile([128, H], bf16, tag="sT")
                nc.vector.tensor_copy(out=st[:, :128], in_=psa[:])
                nc.vector.tensor_copy(out=st[:, 128:], in_=psb[:])
                nc.tensor.matmul(pso[:], lhsT=st[:, :128], rhs=w16[:, :H],
                                 start=False, stop=False)
                nc.tensor.matmul(pso[:], lhsT=st[:, 128:], rhs=w16[:, H:],
                                 start=False, stop=True)
                fl = flp.tile([128, H], f32, tag="flush")
                nc.scalar.copy(out=fl[:nn, :], in_=pso[:nn, :])
                nc.scalar.dma_start(out=outp[n0:n0 + nn, :], in_=fl[:nn, :])

            for step in range(N_WIN + LAG):
                if step < N_WIN:
                    issue_dmas(step)
                if step >= LAG:
                    w = step - LAG
                    compute_ef(w)
                    if w > 0:
                        transform(w - 1)
                    compute_g(w)
            transform(N_WIN - 1)

    nc.compile()
    return nc


def kernel(node_feats, edge_feats, W, b, dst, nbr, eid):
    global _last_in_maps
    node_feats = np.ascontiguousarray(np.asarray(node_feats, dtype=np.float32))
    edge_feats = np.ascontiguousarray(np.asarray(edge_feats, dtype=np.float32))
    W = np.ascontiguousarray(np.asarray(W, dtype=np.float32))
    b = np.asarray(b, dtype=np.float32).reshape(1, H)
    dst = np.asarray(dst, dtype=np.int32)
    nbr = np.asarray(nbr, dtype=np.int32)
    eid = np.asarray(eid, dtype=np.int32)

    T, c0, S, per_core = _build_schedule(dst, nbr)

    key = (S, T.tobytes())
    if key not in _cache:
        _cache.clear()
        _cache[key] = _build_program(T, c0, S)
    nc = _cache[key]

    bf = ml_dtypes.bfloat16
    nf8_arr = node_feats.astype(F8NP)
    ident = np.zeros((128, 128), dtype=F8NP)
    np.fill_diagonal(ident, F8NP(1.0))
    W16 = W.astype(bf)
    b16 = b.astype(bf)

    in_maps = []
    for k in range(N_CORES):
        e0, e1, dk, nk, p, c = per_core[k]
        ef_arr = np.zeros((128, S, H), dtype=F8NP)
        ef_arr[p, c] = edge_feats[eid[e0:e1]].astype(F8NP)
        oh_arr = np.zeros((128, S, 128), dtype=F8NP)
        oh_arr[p, c, dk & 127] = F8NP(1.0)
        # gather indices: slot-within-group j = (c - group_base)*128 + p,
        # encoded at [j%16, 8*group_base + j//16]; unused trailing slots
        # stay -1 so the count-register gather skips them
        gidx_arr = np.full((16, 8 * S), -1, dtype=np.int16)
        w_arr = dk >> 7
        q_arr = (nk >= SPLIT).astype(np.int64)
        idx_val = np.where(q_arr == 0, nk, nk - SPLIT)
        base_col = c0[w_arr, q_arr]
        j = (c - base_col) * 128 + p
        gidx_arr[j % 16, 8 * base_col + j // 16] = idx_val.astype(np.int16)
        gidx_full = np.tile(gidx_arr, (8, 1))
        cnt_arr = np.zeros((1, 2 * N_WIN), dtype=np.int32)
        np.add.at(cnt_arr[0], w_arr * 2 + q_arr, 1)
        deg1_arr = np.zeros((1, N_WIN * WIN), dtype=np.float32)
        deg1_arr[0, :NODES_PER_CORE] = 1.0
        np.add.at(deg1_arr[0], dk, 1.0)
        # own-node fp8 rows: self8[p, w*H + f] = nf8 row of node w*128+p
        own8 = np.zeros((N_WIN * WIN, H), dtype=F8NP)
        own8[:NODES_PER_CORE] = nf8_arr[k * NODES_PER_CORE:(k + 1) * NODES_PER_CORE]
        self8_arr = np.ascontiguousarray(
            own8.reshape(N_WIN, WIN, H).transpose(1, 0, 2).reshape(WIN, N_WIN * H)
        )
        in_maps.append({
            "nf8": nf8_arr,
            "ident": ident,
            "W16": W16,
            "b16": b16,
            "deg1": deg1_arr.astype(bf),
            "self8": self8_arr,
            "ef": ef_arr,
            "oh": oh_arr.reshape(128, S * 128),
            "cnt": cnt_arr,
            "gidx": gidx_full,
        })

    _last_in_maps = in_maps
    res = run_bass_kernel_spmd(nc, in_maps, list(range(N_CORES)))
    out = np.concatenate([res.results[k]["out"] for k in range(N_CORES)], axis=0)
    return out
